# revision 28
# baseline (speedup 1.0000x reference)
"""LSTM rollout kernel for Trainium2 (8 NeuronCores).

Strategy:
  - The LSTM recurrence (B=64 rows) is replicated on all 8 cores (gate matmuls
    are moving-dim bound, so the batch replication costs nothing).
  - Wo / bo / gumbel noise are sharded over the vocab (1250 columns per core).
  - Sampling (jax.random.categorical == argmax(logits + gumbel)) is done by
    computing each core's local top-1 over its vocab slice, AllGather-ing the
    8x2 (value, global index) candidates, and taking the min global index among
    the value-maximal candidates (== jnp.argmax tie semantics).
  - Gumbel noise is precomputed host-side with jax on CPU, bit-identical to
    what the reference's categorical adds.
  - Teacher-forced steps (t < given_num) skip logits/sampling entirely; their
    input embeddings are precomputed host-side (transposed layout).
  - All state is kept transposed (h^T, c^T as [128, 4, 64]) so gate matmuls
    run with M=128 (full PE width, z^T = W^T x^T + U^T h^T) and no per-step
    PE transposes are needed; h^T is directly the lhsT for both the U-matmuls
    and the logits matmul.
"""

import numpy as np

B, T, V, E, H = 64, 32, 10000, 256, 512
N_CORES = 8
VS = V // N_CORES  # 1250 vocab columns per core
VH = VS // 2  # 625, vocab slice half per partition group
ZF = 4 * H  # 2048
MB = ZF // 128  # 16 m-blocks of the transposed gate output
KU = H // 128  # 4 k-tiles for U
KX = E // 128  # 2 k-tiles for W
BIG = 1.0e9

_CACHE = {}
_NOISE_CACHE = {}


def _build(GN):
    import concourse.bacc as bacc
    import concourse.mybir as mybir
    import concourse.tile as tile
    from concourse.bass import IndirectOffsetOnAxis
    from concourse.masks import make_identity

    dt = mybir.dt
    AF = mybir.ActivationFunctionType
    TS = T - GN  # number of sampled steps

    nc = bacc.Bacc(num_devices=N_CORES)

    # ---- I/O ----
    # W layout: [p, k, m, j] = W_all[k*128+p, m*128+j]  (W_all = [Wi|Wf|Wog|Wc])
    w_in = nc.dram_tensor("w_in", [128, KX, MB, 128], dt.float32, kind="ExternalInput")
    u_in = nc.dram_tensor("u_in", [128, KU, MB, 128], dt.float32, kind="ExternalInput")
    # Wo slice layout: [p, k, j] = Wo[k*128+p, core_off + j]
    wo_in = nc.dram_tensor("wo_in", [128, KU, VS], dt.float32, kind="ExternalInput")
    emb_in = nc.dram_tensor("emb_in", [V, E], dt.float32, kind="ExternalInput")
    # teacher x^T: [p, t, k, b] = emb[tok_t[b]][k*128+p]
    xt_in = nc.dram_tensor(
        "xt_in", [128, GN + 1, KX, B], dt.float32, kind="ExternalInput"
    )
    base_in = nc.dram_tensor("base_in", [128, 1], dt.float32, kind="ExternalInput")
    if TS > 0:
        # noise: [p, ts, j] = gumbel[GN+ts, b= p%64, core_off + (p//64)*625 + j] + bo[...]
        noise_in = nc.dram_tensor(
            "noise_in", [128, TS, VH], dt.float32, kind="ExternalInput"
        )
    toks_out = nc.dram_tensor(
        "toks_out", [B, max(TS, 1)], dt.int32, kind="ExternalOutput"
    )

    with tile.TileContext(nc) as tc:
        with (
            tc.tile_pool(name="sb", bufs=1) as sb,
            tc.tile_pool(name="sbw", bufs=2) as sbw,
            tc.tile_pool(name="pz", bufs=1, space="PSUM") as pz,
            tc.tile_pool(name="pzx", bufs=1, space="PSUM") as pzx,
            tc.tile_pool(name="pl", bufs=1, space="PSUM") as pl,
            tc.tile_pool(name="pt", bufs=1, space="PSUM") as pt,
            tc.tile_pool(name="pd", bufs=1, space="PSUM") as pd,
            tc.tile_pool(name="dram", bufs=2, space="DRAM") as dr,
        ):
            # ---- persistent SBUF (load order: what t=0 needs comes first) ----
            w_sb = sb.tile([128, KX, MB, 128], dt.float32)
            xt_sb = sb.tile([128, GN + 1, KX, B], dt.float32)
            nc.sync.dma_start(out=xt_sb[:, 0:1], in_=xt_in[:, 0:1])
            nc.sync.dma_start(out=w_sb[:], in_=w_in[:])
            if GN > 0:
                nc.sync.dma_start(out=xt_sb[:, 1:], in_=xt_in[:, 1:])
            ident = sb.tile([128, 128], dt.float32)
            make_identity(nc, ident[:])
            base_sb = sb.tile([128, 1], dt.float32)
            nc.sync.dma_start(out=base_sb[:], in_=base_in[:])
            u_sb = sb.tile([128, KU, MB, 128], dt.float32)
            nc.sync.dma_start(out=u_sb[:], in_=u_in[:])
            wo_sb = sb.tile([128, KU, VS], dt.float32)
            nc.sync.dma_start(out=wo_sb[:], in_=wo_in[:])
            if TS > 0:
                noise_sb = sb.tile([128, TS, VH], dt.float32)
                nc.sync.dma_start(out=noise_sb[:], in_=noise_in[:])

            # transposed state
            hT = sb.tile([128, KU, B], dt.float32)
            cT = sb.tile([128, KU, B], dt.float32)
            nc.vector.memset(cT[:], 0.0)
            xT = sb.tile([128, KX, B], dt.float32)
            toks_acc = sb.tile([B, max(TS, 1)], dt.int32)
            nc.vector.memset(toks_acc[:], 0)

            def emit_zx(xT_t):
                """x-part of the gates: complete PSUM groups per m-block."""
                zx_ps = pzx.tile([128, MB, B], dt.float32, tag="zx")
                for m in range(MB):
                    for k in range(KX):
                        nc.tensor.matmul(
                            zx_ps[:, m, :],
                            lhsT=w_sb[:, k, m, :],
                            rhs=xT_t[:, k, :],
                            start=(k == 0),
                            stop=(k == KX - 1),
                        )
                return zx_ps

            def emit_zu():
                """h-part of the gates (reads current hT)."""
                zu_ps = pz.tile([128, MB, B], dt.float32, tag="zmain")
                for m in range(MB):
                    for k in range(KU):
                        nc.tensor.matmul(
                            zu_ps[:, m, :],
                            lhsT=u_sb[:, k, m, :],
                            rhs=hT[:, k, :],
                            start=(k == 0),
                            stop=(k == KU - 1),
                        )
                return zu_ps

            def warm_pe(n, rhs):
                """Matmuls that keep the PE's HAM clock warm through windows
                where the real work is blocked (the post-token cell chain).
                `rhs` (an SBUF AP) both feeds the matmul and pins when the
                scheduler can start them; reads delay nothing downstream."""
                fs = 1
                for s in rhs.shape[1:]:
                    fs *= s
                scratch = pd.tile([128, 512], dt.float32)
                for _ in range(n):
                    nc.tensor.matmul(
                        scratch[:, 0:fs],
                        lhsT=w_sb[:, 0, 0, :],
                        rhs=rhs,
                        start=True,
                        stop=True,
                    )

            def stage_to_sbuf(ps):
                """Copy a PSUM tile to SBUF on the (otherwise idle) ACT
                engine so the final DVE add has only one PSUM operand."""
                sb_t = sbw.tile([128, MB, B], dt.float32, tag="zstage")
                nc.scalar.copy(sb_t[:], ps[:])
                return sb_t

            # zU of the next step is emitted during the previous step's
            # AllGather window (PE is in-order; this keeps it busy there).
            pending_zu_sb = None

            for t in range(T):
                # ---- gates z^T [128, 16, 64]: m-block m = gate dims m*128..
                # blocks 0:4 = i, 4:8 = f, 8:12 = o, 12:16 = c
                # PSUM accumulation groups must be contiguous per (bank,
                # partitions): a later start=True on the same bank clears
                # has_written for its partitions, so an interleaved group's
                # start=False matmuls would overwrite instead of accumulate.
                # The x-part and h-part therefore live in separate PSUM tiles;
                # the early-available one is staged to SBUF and added to the
                # late one with a single DVE op.
                if t == 0:
                    zx_ps = emit_zx(xt_sb[:, 0, :, :])
                elif t <= GN:
                    # teacher x known: emit zx first so it fills the PE gap
                    # while the previous step's ACT/DVE cell chain runs.
                    zx_ps = emit_zx(xt_sb[:, t, :, :])
                    zx_sb = stage_to_sbuf(zx_ps)
                    zu_ps = emit_zu()
                else:
                    # sampled x: zU was emitted during the previous step's
                    # AllGather; zx waits on the gathered embedding.
                    zx_ps = emit_zx(xT[:])

                # ---- per-gate add + activation (pipelined), then cell ----
                # gate blocks: i = 0:KU, f = KU:2KU, o = 2KU:3KU, c = 3KU:4KU
                sig = sbw.tile([128, 3 * KU, B], dt.float32)
                ctil = sbw.tile([128, KU, B], dt.float32)
                if t == 0:
                    nc.scalar.activation(
                        sig[:], zx_ps[:, 0 : 3 * KU, :], AF.Sigmoid
                    )
                    nc.scalar.activation(
                        ctil[:], zx_ps[:, 3 * KU : 4 * KU, :], AF.Tanh
                    )
                else:
                    if t <= GN:
                        late, early = zu_ps, zx_sb
                    else:
                        late, early = zx_ps, pending_zu_sb
                    zs = sbw.tile([128, MB, B], dt.float32)
                    first = True
                    for g in (1, 3, 0, 2):  # f, c, i, o
                        blk = slice(g * KU, (g + 1) * KU)
                        nc.vector.tensor_add(
                            zs[:, blk, :], late[:, blk, :], early[:, blk, :]
                        )
                        if g == 3:
                            nc.scalar.activation(ctil[:], zs[:, blk, :], AF.Tanh)
                        else:
                            nc.scalar.activation(
                                sig[:, blk, :], zs[:, blk, :], AF.Sigmoid
                            )
                        if first and t >= GN:
                            # keep PE warm through the cell chain; pinned
                            # after the first gate add via the zs read
                            warm_pe(7, zs[:, KU : 2 * KU, :])
                            first = False

                # ---- cell update (all transposed [128, 4, 64]) ----
                fc = sbw.tile([128, KU, B], dt.float32)
                nc.vector.tensor_mul(fc[:], sig[:, KU : 2 * KU, :], cT[:])
                ic = sbw.tile([128, KU, B], dt.float32)
                nc.vector.tensor_mul(ic[:], sig[:, 0:KU, :], ctil[:])
                nc.vector.tensor_add(cT[:], fc[:], ic[:])
                th = sbw.tile([128, KU, B], dt.float32)
                nc.scalar.activation(th[:], cT[:], AF.Tanh)
                nc.vector.tensor_mul(hT[:], sig[:, 2 * KU : 3 * KU, :], th[:])

                if t < GN:
                    continue

                ts = t - GN

                # ---- logits over this core's vocab slice: [128, 625] ----
                # partitions 0:64 = batch x cols [0,625); 64:128 = cols [625,1250)
                l_ps = pl.tile([128, VH], dt.float32)
                for k in range(KU):
                    for half in range(2):
                        for n0, n1 in ((0, 512), (512, VH)):
                            nc.tensor.matmul(
                                l_ps[64 * half : 64 * (half + 1), n0:n1],
                                lhsT=hT[:, k, :],
                                rhs=wo_sb[:, k, VH * half + n0 : VH * half + n1],
                                start=(k == 0),
                                stop=(k == KU - 1),
                                tile_position=(0, 64 * half),
                            )

                # ---- scores = logits + gumbel noise ----
                scores = sbw.tile([128, VH], dt.float32)
                nc.vector.tensor_add(scores[:], l_ps[:], noise_sb[:, ts, :])

                # ---- local top-1 (per partition-group) ----
                v8 = sbw.tile([128, 8], dt.float32)
                nc.vector.max(out=v8[:], in_=scores[:])
                i8 = sbw.tile([128, 8], dt.uint32)
                nc.vector.max_index(out=i8[:], in_max=v8[:], in_values=scores[:])
                cand = sbw.tile([128, 2], dt.float32)
                nc.vector.tensor_copy(cand[:, 0:1], v8[:, 0:1])
                nc.vector.tensor_scalar(
                    out=cand[:, 1:2], in0=i8[:, 0:1], scalar1=base_sb[:, 0:1],
                    scalar2=None, op0=mybir.AluOpType.add,
                )

                # ---- AllGather candidates ----
                ag_in = dr.tile([2 * 128], dt.float32)
                nc.sync.dma_start(
                    out=ag_in[:].rearrange("(two p) -> p two", two=2, p=128),
                    in_=cand[:],
                )
                ag_out = dr.tile([N_CORES * 2 * 128], dt.float32)
                nc.gpsimd.collective_compute(
                    "AllGather",
                    mybir.AluOpType.bypass,
                    replica_groups=[list(range(N_CORES))],
                    ins=[ag_in[:].opt()],
                    outs=[ag_out[:].opt()],
                )

                # next step's zU: fills the PE during the AllGather (must be
                # emitted before the gather-dependent transposes, PE is
                # in-order)
                if t + 1 < T:
                    pending_zu_sb = stage_to_sbuf(emit_zu())
                    warm_pe(4, pending_zu_sb[:, 0:8, :])

                ag_view = ag_out[:].rearrange(
                    "(r two half b) -> b r two half", r=N_CORES, two=2, half=2, b=B
                )
                valT = sbw.tile([B, N_CORES, 2], dt.float32)
                idxT = sbw.tile([B, N_CORES, 2], dt.float32)
                for half in range(2):
                    nc.sync.dma_start(
                        out=valT[:, :, half], in_=ag_view[:, :, 0, half]
                    )
                    nc.gpsimd.dma_start(
                        out=idxT[:, :, half], in_=ag_view[:, :, 1, half]
                    )

                # ---- global argmax: min global idx among value-maximal ----
                g8 = sbw.tile([B, 8], dt.float32)
                nc.vector.max(out=g8[:], in_=valT[:])
                mask = sbw.tile([B, N_CORES, 2], dt.uint32)
                nc.vector.tensor_scalar(
                    out=mask[:], in0=valT[:], scalar1=g8[:, 0:1], scalar2=None,
                    op0=mybir.AluOpType.is_equal,
                )
                sel = sbw.tile([B, N_CORES, 2], dt.float32)
                nc.vector.memset(sel[:], BIG)
                nc.vector.copy_predicated(sel[:], mask[:], idxT[:])
                tokf = sbw.tile([B, 1], dt.float32)
                nc.vector.tensor_reduce(
                    out=tokf[:], in_=sel[:], axis=mybir.AxisListType.XY,
                    op=mybir.AluOpType.min,
                )
                toki = sbw.tile([B, 1], dt.int32)
                nc.vector.tensor_copy(toki[:], tokf[:])
                nc.vector.tensor_copy(toks_acc[:, ts : ts + 1], toki[:])

                # ---- embedding gather + transpose for next step ----
                if t + 1 < T:
                    x_sb = sbw.tile([B, E], dt.float32)
                    nc.gpsimd.indirect_dma_start(
                        out=x_sb[:],
                        out_offset=None,
                        in_=emb_in[:],
                        in_offset=IndirectOffsetOnAxis(ap=toki[:, :1], axis=0),
                    )
                    for k in range(KX):
                        p_tr = pt.tile([128, B], dt.float32)
                        nc.tensor.transpose(
                            p_tr[:],
                            x_sb[:, 128 * k : 128 * (k + 1)],
                            ident[:B, :B],
                        )
                        nc.vector.tensor_copy(xT[:, k, :], p_tr[:])

            nc.sync.dma_start(out=toks_out[:], in_=toks_acc[:])

    nc.compile()
    return nc


def _prep_inputs(inputs, GN):
    """Host-side input prep. Returns per-core input maps."""
    import jax
    import jax.numpy as jnp

    TS = T - GN
    input_x = np.asarray(inputs["input_x"], dtype=np.int32)
    start_token = np.asarray(inputs["start_token"], dtype=np.int32)
    g = np.ascontiguousarray(np.asarray(inputs["g_embeddings"], dtype=np.float32))
    f32 = np.float32

    W_all = np.concatenate(
        [np.asarray(inputs[n], dtype=f32) for n in ("Wi", "Wf", "Wog", "Wc")], axis=1
    )  # [E, 2048]
    U_all = np.concatenate(
        [np.asarray(inputs[n], dtype=f32) for n in ("Ui", "Uf", "Uog", "Uc")], axis=1
    )  # [H, 2048]
    b_all = np.concatenate(
        [np.asarray(inputs[n], dtype=f32) for n in ("bi", "bf", "bog", "bc")]
    )
    assert np.abs(b_all).max() == 0.0, "nonzero gate biases not supported"
    Wo = np.asarray(inputs["Wo"], dtype=f32)  # [H, V]
    bo = np.asarray(inputs["bo"], dtype=f32)  # [V]

    # [p, k, m, j] = W_all[k*128+p, m*128+j]
    w_host = np.ascontiguousarray(
        W_all.reshape(KX, 128, MB, 128).transpose(1, 0, 2, 3)
    )
    u_host = np.ascontiguousarray(
        U_all.reshape(KU, 128, MB, 128).transpose(1, 0, 2, 3)
    )

    # teacher-phase x^T: steps 0..GN use host-known tokens
    toks = np.concatenate([start_token[None, :], input_x[:, :GN].T], axis=0)  # [GN+1,B]
    X = g[toks]  # [GN+1, B, E]
    xt_host = np.ascontiguousarray(
        X.transpose(2, 0, 1).reshape(KX, 128, GN + 1, B).transpose(1, 2, 0, 3)
    )  # [128, GN+1, 2, B]

    # gumbel noise, bit-identical to the reference's categorical on CPU
    if TS > 0:
        if GN in _NOISE_CACHE:
            gumbel = _NOISE_CACHE[GN]
        else:
            cpu = jax.devices("cpu")[0]
            with jax.default_device(cpu):
                keys = jax.random.split(jax.random.key(42), T)
                gumbel = np.stack(
                    [
                        np.asarray(jax.random.gumbel(keys[t], (B, V), jnp.float32))
                        for t in range(GN, T)
                    ]
                )  # [TS, B, V]
            _NOISE_CACHE[GN] = gumbel
        noise = gumbel + bo[None, None, :]

    in_maps = []
    for c in range(N_CORES):
        cs = c * VS
        wo_host = np.ascontiguousarray(
            Wo[:, cs : cs + VS].reshape(KU, 128, VS).transpose(1, 0, 2)
        )
        base = np.zeros((128, 1), f32)
        base[:64, 0] = cs
        base[64:, 0] = cs + VH
        m = {
            "w_in": w_host,
            "u_in": u_host,
            "wo_in": wo_host,
            "emb_in": g,
            "xt_in": xt_host,
            "base_in": base,
        }
        if TS > 0:
            ns = noise[:, :, cs : cs + VS].reshape(TS, B, 2, VH)
            m["noise_in"] = np.ascontiguousarray(
                ns.transpose(2, 1, 0, 3).reshape(128, TS, VH)
            )
        in_maps.append(m)
    return in_maps


def kernel(**inputs):
    from concourse.bass_utils import run_bass_kernel_spmd

    GN = int(np.asarray(inputs["given_num"]))
    GN = max(0, min(GN, T))
    input_x = np.asarray(inputs["input_x"], dtype=np.int32)
    if GN >= T:
        return input_x.copy()

    if GN not in _CACHE:
        _CACHE[GN] = _build(GN)
    nc = _CACHE[GN]

    in_maps = _prep_inputs(inputs, GN)
    res = run_bass_kernel_spmd(nc, in_maps, core_ids=list(range(N_CORES)))
    toks = res.results[0]["toks_out"]  # [B, TS] int32
    out = np.concatenate([input_x[:, :GN], toks[:, : T - GN]], axis=1).astype(np.int32)
    return out
